# revision 10
# baseline (speedup 1.0000x reference)
"""Trainium2 Bass kernel for a GQA attention block (RMSNorm -> QKV+gate ->
Q/K-norm -> RoPE -> attention -> gated out -> proj), tensor-parallel over
heads across 8 NeuronCores.

Sharding: core c owns q heads [5c, 5c+5) and kv group c (NQ=40, NKV=8).
Each core computes a partial projection output; partials are summed on host
(row-parallel proj unshard).
"""
import sys

sys.path.insert(0, "/opt/trn_rl_repo")

import numpy as np

import concourse.bacc as bacc
import concourse.tile as tile
from concourse import mybir

NQ, NKV, D, HID = 40, 8, 128, 5120
S = 2048
NC = 8
HPC = NQ // NC          # q heads per core = 5
EPS = 1e-6
HT = HID // 128         # 40 hid tiles
ST = S // 128           # 16 seq tiles of 128
NCH = S // 512          # 4 chunks of 512
KT = S // 128           # 16 k-tiles
QKV_COLS = HPC * D + 2 * D + HPC   # 901
F32 = mybir.dt.float32
F32R = mybir.dt.float32r
AF = mybir.ActivationFunctionType


def build_program():
    nc = bacc.Bacc(None, target_bir_lowering=False)

    # register activation-bias constants (mirrors Bass.__init__ registration)
    for val in (EPS, float(D) * EPS):
        t = nc.alloc_sbuf_tensor(f"const-float32-{val}", [128, 1], F32)
        nc.gpsimd.memset(t.ap(), val)
        nc.const_aps.aps[(F32, val)] = t.ap()
    nc.all_engine_barrier()

    # ---- I/O ----
    xT = nc.dram_tensor("xT", [HT, 128, S], F32R, kind="ExternalInput")
    wq = nc.dram_tensor("wq", [HT, 128, QKV_COLS], F32R, kind="ExternalInput")
    wp = nc.dram_tensor("wp", [HPC, 128, HID], F32R, kind="ExternalInput")
    cosq = nc.dram_tensor("cosq", [128, S], F32, kind="ExternalInput")
    sinq = nc.dram_tensor("sinq", [128, S], F32, kind="ExternalInput")
    cosk = nc.dram_tensor("cosk", [128, S], F32, kind="ExternalInput")
    sink = nc.dram_tensor("sink", [128, S], F32, kind="ExternalInput")
    ones_col = nc.dram_tensor("ones_col", [128, 1], F32R, kind="ExternalInput")
    ident = nc.dram_tensor("ident", [128, 128], F32R, kind="ExternalInput")
    out = nc.dram_tensor("out", [S, HID], F32, kind="ExternalOutput")

    with tile.TileContext(nc) as tc:
        with tc.tile_pool(name="persist", bufs=1) as pers, \
             tc.tile_pool(name="cols", bufs=1) as cols, \
             tc.tile_pool(name="scr", bufs=1, space="DRAM") as dscr:
            # DRAM row bounces (for partition-broadcast / row->col reshape)
            invr_scr = dscr.tile([1, S], F32, name="invr_scr")
            rrow_scr = dscr.tile([1, S], F32, name="rrow_scr")
            nq_scr = dscr.tile([HPC, S], F32, name="nq_scr")
            nk_scr = dscr.tile([1, S], F32, name="nk_scr")
            gate_scr = dscr.tile([HPC, S], F32, name="gate_scr")
            # persistent tiles
            t_ones = cols.tile([128, 1], F32R, name="ones")
            nc.sync.dma_start(t_ones[:, :], ones_col[:, :])
            t_id = cols.tile([128, 128], F32R, name="ident")
            nc.sync.dma_start(t_id[:, :], ident[:, :])

            q_t = [pers.tile([128, S], F32R, name=f"q{h}") for h in range(HPC)]
            k_t = pers.tile([128, S], F32R, name="kT")
            v_t = pers.tile([128, S], F32R, name="vT")
            vnat = pers.tile([128, S], F32R, name="vnat")
            gates = pers.tile([5, S], F32, name="gates")
            o_t = [pers.tile([128, S], F32R, name=f"o{h}") for h in range(HPC)]

            # ================= Phase 1: QKV matmul (+ x^2 accumulation) ========
            with tc.tile_pool(name="qkv_ps", bufs=1, space="PSUM") as psA, \
                 tc.tile_pool(name="qkv_psB", bufs=1, space="PSUM") as psB, \
                 tc.tile_pool(name="qkv_sb", bufs=2) as sb2, \
                 tc.tile_pool(name="qkv_sb3", bufs=3) as sb3:
                for ch in range(NCH):
                    c0 = ch * 512
                    pm = [psA.tile([128, 512], F32, name=f"mt{m}") for m in range(7)]
                    pg = psB.tile([5, 512], F32, name="small")
                    acc = sb2.tile([128, 512], F32, name="accsq")
                    accr = sb2.tile([128, 512], F32R, name="accr")
                    for ht in range(HT):
                        xt = sb2.tile([128, 512], F32R, name="xt")
                        nc.sync.dma_start(xt[:, :], xT[ht, :, c0:c0 + 512])
                        ws = sb2.tile([128, QKV_COLS], F32R, name="wslab")
                        nc.sync.dma_start(ws[:, :], wq[ht, :, :])
                        for m in range(7):
                            nc.tensor.matmul(pm[m][:, :], ws[:, m * 128:(m + 1) * 128],
                                             xt[:, :], start=(ht == 0), stop=(ht == HT - 1))
                        nc.tensor.matmul(pg[:, :], ws[:, 896:901], xt[:, :],
                                         start=(ht == 0), stop=(ht == HT - 1))
                        # x^2 accumulation (for pre-norm rms)
                        sq = sb3.tile([128, 512], F32, name="sq")
                        nc.scalar.activation(sq[:, :], xt[:, :].bitcast(F32), AF.Square)
                        if ht == 0:
                            nc.gpsimd.tensor_copy(acc[:, :], sq[:, :])
                        elif ht < HT - 1:
                            nc.gpsimd.tensor_add(acc[:, :], acc[:, :], sq[:, :])
                        else:
                            # last add on DVE, writes rounded f32r tile
                            nc.vector.tensor_add(accr[:, :], acc[:, :], sq[:, :])
                    # drain psums
                    for h in range(HPC):
                        nc.vector.tensor_copy(q_t[h][:, c0:c0 + 512], pm[h][:, :])
                    nc.vector.tensor_copy(k_t[:, c0:c0 + 512], pm[5][:, :])
                    nc.vector.tensor_copy(v_t[:, c0:c0 + 512], pm[6][:, :])
                    nc.vector.tensor_copy(gates[:, c0:c0 + 512], pg[:, :])
                    # r = 1/sqrt(mean_h x^2 + eps); bounce rows to DRAM
                    pr = psB.tile([1, 512], F32, name="small")
                    nc.tensor.matmul(pr[:, :], t_ones[:, :], accr[:, :],
                                     start=True, stop=True)
                    invr_row = sb2.tile([1, 512], F32, name="invr_row")
                    nc.scalar.activation(invr_row[:, :], pr[:, :], AF.Sqrt,
                                         bias=EPS, scale=1.0 / HID)
                    nc.sync.dma_start(invr_scr[0:1, c0:c0 + 512], invr_row[:, :])
                    r_row = sb2.tile([1, 512], F32, name="r_row")
                    nc.vector.reciprocal(r_row[:, :], invr_row[:, :])
                    nc.sync.dma_start(rrow_scr[0:1, c0:c0 + 512], r_row[:, :])

            # ================= Phase 2: prep (gates, v transpose, k/q norm+rope)
            with tc.tile_pool(name="at_sc", bufs=3, space="PSUM") as pSC, \
                 tc.tile_pool(name="at_av", bufs=2, space="PSUM") as pAV, \
                 tc.tile_pool(name="at_row", bufs=2, space="PSUM") as pRow, \
                 tc.tile_pool(name="at_tr", bufs=1, space="PSUM") as pTr, \
                 tc.tile_pool(name="at_sb2", bufs=2) as asb2, \
                 tc.tile_pool(name="at_sb3", bufs=3) as asb3:

                # gates: g = sigmoid(r * g_raw)
                for ch in range(NCH):
                    c0 = ch * 512
                    rb = asb2.tile([5, 512], F32, name="rhatb")
                    nc.sync.dma_start(rb[:, :],
                                      rrow_scr[0:1, c0:c0 + 512].to_broadcast((5, 512)))
                    nc.vector.tensor_mul(gates[:, c0:c0 + 512],
                                         gates[:, c0:c0 + 512], rb[:, :])
                nc.scalar.activation(gates[:, :], gates[:, :], AF.Sigmoid)
                nc.sync.dma_start(gate_scr[:, :], gates[:, :])

                # v transpose: vnat[:, kt*128:(kt+1)*128] = v block kt (natural [kpos, D])
                for kt in range(KT):
                    ptr = pTr.tile([128, 128], F32R, name="tr")
                    nc.tensor.transpose(ptr[:, :], v_t[:, kt * 128:(kt + 1) * 128],
                                        t_id[:, :])
                    nc.vector.tensor_copy(vnat[:, kt * 128:(kt + 1) * 128], ptr[:, :])

                # ---- k: norm sums, rope (no n_k application; folded into exp scale)
                for ch in range(NCH):
                    c0 = ch * 512
                    ksq = asb3.tile([128, 512], F32R, name="sqc")
                    nc.scalar.activation(ksq[:, :], k_t[:, c0:c0 + 512].bitcast(F32),
                                         AF.Square)
                    pn = pRow.tile([1, 512], F32, name="nrow")
                    nc.tensor.matmul(pn[:, :], t_ones[:, :], ksq[:, :],
                                     start=True, stop=True)
                    nk_row = asb2.tile([1, 512], F32, name="nkrow")
                    # 1/(n_k/sqrt(D)) = sqrt(sum + D*eps)
                    nc.scalar.activation(nk_row[:, :], pn[:, :], AF.Sqrt,
                                         bias=D * EPS, scale=1.0)
                    nc.sync.dma_start(nk_scr[0:1, c0:c0 + 512], nk_row[:, :])

                t_cos = pers.tile([128, S], F32, name="cosT")
                t_sin = pers.tile([128, S], F32, name="sinT")
                nc.sync.dma_start(t_cos[:, :], cosk[:, :])
                nc.sync.dma_start(t_sin[:, :], sink[:, :])
                for ch in range(NCH):
                    c0 = ch * 512
                    rot = asb2.tile([128, 512], F32, name="rot")
                    nc.sync.dma_start(rot[0:64, :],
                                      k_t[64:128, c0:c0 + 512].bitcast(F32))
                    nc.sync.dma_start(rot[64:128, :],
                                      k_t[0:64, c0:c0 + 512].bitcast(F32))
                    t1 = asb2.tile([128, 512], F32, name="t1")
                    nc.vector.tensor_mul(t1[:, :], k_t[:, c0:c0 + 512].bitcast(F32),
                                         t_cos[:, c0:c0 + 512])
                    nc.vector.tensor_mul(rot[:, :], rot[:, :], t_sin[:, c0:c0 + 512])
                    nc.vector.tensor_add(k_t[:, c0:c0 + 512], t1[:, :], rot[:, :])

                # ---- q heads: norm sums -> nq rows; rope; apply n_q
                for h in range(HPC):
                    for ch in range(NCH):
                        c0 = ch * 512
                        qsq = asb3.tile([128, 512], F32R, name="sqc")
                        nc.scalar.activation(qsq[:, :],
                                             q_t[h][:, c0:c0 + 512].bitcast(F32),
                                             AF.Square)
                        pn = pRow.tile([1, 512], F32, name="nrow")
                        nc.tensor.matmul(pn[:, :], t_ones[:, :], qsq[:, :],
                                         start=True, stop=True)
                        sm = asb2.tile([1, 512], F32, name="sqrtm")
                        nc.scalar.activation(sm[:, :], pn[:, :], AF.Sqrt,
                                             bias=EPS, scale=1.0 / D)
                        nq_row = asb2.tile([1, 512], F32, name="nqrow")
                        nc.vector.reciprocal(nq_row[:, :], sm[:, :])
                        nc.sync.dma_start(nq_scr[h:h + 1, c0:c0 + 512], nq_row[:, :])

                # rope tables for q (overwrites k tables; waits for k-rope reads)
                nc.sync.dma_start(t_cos[:, :], cosq[:, :])
                nc.sync.dma_start(t_sin[:, :], sinq[:, :])
                for h in range(HPC):
                    for ch in range(NCH):
                        c0 = ch * 512
                        rot = asb2.tile([128, 512], F32, name="rot")
                        nc.sync.dma_start(rot[0:64, :],
                                          q_t[h][64:128, c0:c0 + 512].bitcast(F32))
                        nc.sync.dma_start(rot[64:128, :],
                                          q_t[h][0:64, c0:c0 + 512].bitcast(F32))
                        t1 = asb2.tile([128, 512], F32, name="t1")
                        nc.vector.tensor_mul(t1[:, :],
                                             q_t[h][:, c0:c0 + 512].bitcast(F32),
                                             t_cos[:, c0:c0 + 512])
                        nc.vector.tensor_mul(rot[:, :], rot[:, :],
                                             t_sin[:, c0:c0 + 512])
                        nb = asb2.tile([128, 512], F32, name="nb")
                        nc.sync.dma_start(
                            nb[:, :],
                            nq_scr[h:h + 1, c0:c0 + 512].to_broadcast((128, 512)))
                        nc.vector.tensor_add(t1[:, :], t1[:, :], rot[:, :])
                        nc.vector.tensor_mul(q_t[h][:, c0:c0 + 512], t1[:, :], nb[:, :])

                # exp scale/bias columns: nk_col = 1/nk_row (col), lnr = -ln(invr)
                nkc_raw = cols.tile([128, KT], F32, name="nkc_raw")
                nc.sync.dma_start(nkc_raw[:, :],
                                  nk_scr[0, :].rearrange("(t p) -> p t", p=128))
                nk_col = cols.tile([128, KT], F32, name="nk_col")
                nc.vector.reciprocal(nk_col[:, :], nkc_raw[:, :])
                invr_col = cols.tile([128, KT], F32, name="invr_col")
                nc.sync.dma_start(invr_col[:, :],
                                  invr_scr[0, :].rearrange("(t p) -> p t", p=128))
                lnr_col = cols.tile([128, KT], F32, name="lnr_col")
                nc.scalar.activation(lnr_col[:, :], invr_col[:, :], AF.Ln)
                nc.vector.tensor_scalar_mul(lnr_col[:, :], lnr_col[:, :], -1.0)
                rinv_col = cols.tile([128, KT], F32R, name="rinv_col")
                nc.vector.tensor_copy(rinv_col[:, :], invr_col[:, :])

                # ================= Phase 3: attention ======================
                for h in range(HPC):
                    for ch in range(NCH):
                        c0 = ch * 512
                        po = pAV.tile([128, 512], F32, name="av")
                        psum_row = pRow.tile([1, 512], F32, name="nrow")
                        for kt in range(KT):
                            k0 = kt * 128
                            ps = pSC.tile([128, 512], F32, name="sc")
                            nc.tensor.matmul(ps[:, :], k_t[:, k0:k0 + 128],
                                             q_t[h][:, c0:c0 + 512],
                                             start=True, stop=True)
                            et = asb3.tile([128, 512], F32R, name="expt")
                            nc.scalar.activation(et[:, :], ps[:, :], AF.Exp,
                                                 bias=lnr_col[:, kt:kt + 1],
                                                 scale=nk_col[:, kt:kt + 1])
                            nc.tensor.matmul(po[:, :], vnat[:, k0:k0 + 128], et[:, :],
                                             start=(kt == 0), stop=(kt == KT - 1))
                            nc.tensor.matmul(psum_row[:, :], rinv_col[:, kt:kt + 1],
                                             et[:, :], start=(kt == 0),
                                             stop=(kt == KT - 1))
                        # scale = gate / sum ; outT = po * scale
                        rcp = asb2.tile([1, 512], F32, name="rcp")
                        nc.vector.reciprocal(rcp[:, :], psum_row[:, :])
                        grow = asb2.tile([1, 512], F32, name="grow")
                        nc.sync.dma_start(grow[:, :], gate_scr[h:h + 1, c0:c0 + 512])
                        nc.vector.tensor_mul(rcp[:, :], rcp[:, :], grow[:, :])
                        sb = asb2.tile([128, 512], F32, name="scaleb")
                        nc.gpsimd.partition_broadcast(sb[:, :], rcp[:, :])
                        nc.vector.tensor_mul(o_t[h][:, c0:c0 + 512], po[:, :],
                                             sb[:, :])

            # ================= Phase 4: projection =====================
            with tc.tile_pool(name="pj_ps", bufs=4, space="PSUM") as pPJ, \
                 tc.tile_pool(name="pj_sb", bufs=2) as pjs:
                NT = HID // 512  # 10
                for nt in range(NT):
                    n0 = nt * 512
                    wt = [pjs.tile([128, 512], F32R, name=f"wp{h}") for h in range(HPC)]
                    for h in range(HPC):
                        nc.sync.dma_start(wt[h][:, :], wp[h, :, n0:n0 + 512])
                    for st in range(ST):
                        s0 = st * 128
                        pp = pPJ.tile([128, 512], F32, name="pj")
                        for h in range(HPC):
                            nc.tensor.matmul(pp[:, :], o_t[h][:, s0:s0 + 128],
                                             wt[h][:, :], start=(h == 0),
                                             stop=(h == HPC - 1))
                        ob = pjs.tile([128, 512], F32, name="outsb")
                        nc.vector.tensor_copy(ob[:, :], pp[:, :])
                        nc.sync.dma_start(out[s0:s0 + 128, n0:n0 + 512], ob[:, :])
    nc.finalize()
    return nc


# ---------------- host-side prep & execution ----------------

_CACHE = {}


def _get_exec():
    if "fn" in _CACHE:
        return _CACHE

    import jax
    from concourse import bass2jax, mybir as mb
    from jax.experimental.shard_map import shard_map
    from jax.sharding import Mesh, PartitionSpec

    bass2jax.install_neuronx_cc_hook()
    nc = build_program()

    part_name = nc.partition_id_tensor.name if nc.partition_id_tensor else None
    in_names, out_names, out_avals = [], [], []
    for alloc in nc.m.functions[0].allocations:
        if not isinstance(alloc, mb.MemoryLocationSet):
            continue
        name = alloc.memorylocations[0].name
        if alloc.kind == "ExternalInput":
            if name != part_name:
                in_names.append(name)
        elif alloc.kind == "ExternalOutput":
            out_names.append(name)
            out_avals.append(jax.core.ShapedArray(tuple(alloc.tensor_shape),
                                                  mb.dt.np(alloc.dtype)))
    n_params = len(in_names)
    all_names = in_names + out_names
    if part_name is not None:
        all_names = all_names + [part_name]

    def _body(*args):
        operands = list(args)
        if part_name is not None:
            operands.append(bass2jax.partition_id_tensor())
        outs = bass2jax._bass_exec_p.bind(
            *operands,
            out_avals=tuple(out_avals),
            in_names=tuple(all_names),
            out_names=tuple(out_names),
            lowering_input_output_aliases=(),
            sim_require_finite=True,
            sim_require_nnan=True,
            nc=nc,
        )
        return tuple(outs)

    devices = jax.devices()[:NC]
    mesh = Mesh(np.asarray(devices), ("core",))
    spec = (PartitionSpec("core"),) * (n_params + len(out_names))
    fn = jax.jit(shard_map(_body, mesh=mesh, in_specs=spec,
                           out_specs=(PartitionSpec("core"),) * len(out_names),
                           check_rep=False), keep_unused=True)
    _CACHE.update(dict(fn=fn, nc=nc, in_names=in_names, out_names=out_names,
                       out_avals=out_avals, mesh=mesh))
    return _CACHE


def prep_inputs(x, rope_cos, rope_sin, w_pre_norm, w_qkv, w_q_norm, w_k_norm,
                w_proj):
    """Build the per-core input dict list (host-side sharding/layout only)."""
    x = np.asarray(x, np.float32)
    w_qkv = np.asarray(w_qkv, np.float32)
    w_proj = np.asarray(w_proj, np.float32)
    w_pre = np.asarray(w_pre_norm, np.float32)
    w_qn = np.asarray(w_q_norm, np.float32)
    w_kn = np.asarray(w_k_norm, np.float32)
    cos = np.asarray(rope_cos, np.float32)[0]   # [S, D]
    sin = np.asarray(rope_sin, np.float32)[0]

    xT = np.ascontiguousarray(x[0].T).reshape(HT, 128, S)

    cosT = np.ascontiguousarray(cos.T)          # [D, S]
    sinT = np.ascontiguousarray(sin.T)
    sign = np.where(np.arange(D) < D // 2, -1.0, 1.0).astype(np.float32)

    def rope_tables(w):
        w_swap = np.concatenate([w[D // 2:], w[:D // 2]])
        c = cosT * w[:, None]
        s = sinT * (sign * w_swap)[:, None]
        return np.ascontiguousarray(c), np.ascontiguousarray(s)

    cq, sq_ = rope_tables(w_qn)
    ck, sk = rope_tables(w_kn)

    wqkv_eff = w_pre[:, None] * w_qkv           # fold pre-norm weight (exact)
    q_dim, k_dim = NQ * D, NKV * D
    ones = np.ones((128, 1), np.float32)
    ident = np.eye(128, dtype=np.float32)

    in_maps = []
    for c in range(NC):
        wslice = np.concatenate([
            wqkv_eff[:, (HPC * c) * D:(HPC * c + HPC) * D],
            wqkv_eff[:, q_dim + c * D:q_dim + (c + 1) * D],
            wqkv_eff[:, q_dim + k_dim + c * D:q_dim + k_dim + (c + 1) * D],
            wqkv_eff[:, q_dim + 2 * k_dim + HPC * c:q_dim + 2 * k_dim + HPC * (c + 1)],
        ], axis=1)                               # [HID, 901]
        wslice = np.ascontiguousarray(wslice).reshape(HT, 128, QKV_COLS)
        wpslice = np.ascontiguousarray(
            w_proj[(HPC * c) * D:(HPC * c + HPC) * D, :]).reshape(HPC, 128, HID)
        in_maps.append({
            "xT": xT, "wq": wslice, "wp": wpslice,
            "cosq": cq, "sinq": sq_, "cosk": ck, "sink": sk,
            "ones_col": ones, "ident": ident,
        })
    return in_maps


def run_in_maps(in_maps):
    """Execute the SPMD program; returns list of per-core {out: [S, HID]}."""
    cache = _get_exec()
    fn, in_names, out_names, out_avals = (cache["fn"], cache["in_names"],
                                          cache["out_names"], cache["out_avals"])
    concat_in = [np.concatenate([m[nm] for m in in_maps], axis=0)
                 for nm in in_names]
    zeros = [np.zeros((NC * a.shape[0], *a.shape[1:]), a.dtype) for a in out_avals]
    outs = fn(*concat_in, *zeros)
    res = []
    for c in range(NC):
        d = {}
        for i, nm in enumerate(out_names):
            shp = out_avals[i].shape
            d[nm] = np.asarray(outs[i]).reshape(NC, *shp)[c]
        res.append(d)
    return res


def kernel(**inputs):
    in_maps = prep_inputs(**inputs)
    res = run_in_maps(in_maps)
    total = res[0]["out"].astype(np.float32)
    for c in range(1, NC):
        total = total + res[c]["out"]
    return total.reshape(1, S, HID)


# revision 11
# speedup vs baseline: 38.5144x; 38.5144x over previous
"""Trainium2 Bass kernel for a GQA attention block (RMSNorm -> QKV+gate ->
Q/K-norm -> RoPE -> attention -> gated out -> proj), tensor-parallel over
heads across 8 NeuronCores.

Sharding: core c owns q heads [5c, 5c+5) and kv group c (NQ=40, NKV=8).
Each core computes a partial projection output; partials are summed on host
(row-parallel proj unshard).
"""
import sys

sys.path.insert(0, "/opt/trn_rl_repo")

import numpy as np

import concourse.bacc as bacc
import concourse.tile as tile
from concourse import mybir

NQ, NKV, D, HID = 40, 8, 128, 5120
S = 2048
NC = 8
HPC = NQ // NC          # q heads per core = 5
EPS = 1e-6
HT = HID // 128         # 40 hid tiles
ST = S // 128           # 16 seq tiles of 128
NCH = S // 512          # 4 chunks of 512
KT = S // 128           # 16 k-tiles
QKV_COLS = HPC * D + 2 * D + HPC   # 901
F32 = mybir.dt.float32
F32R = mybir.dt.float32r
AF = mybir.ActivationFunctionType


def build_program(repeat=1):
    nc = bacc.Bacc(None, target_bir_lowering=False)

    # register activation-bias constants (mirrors Bass.__init__ registration)
    for val in (EPS, float(D) * EPS):
        t = nc.alloc_sbuf_tensor(f"const-float32-{val}", [128, 1], F32)
        nc.gpsimd.memset(t.ap(), val)
        nc.const_aps.aps[(F32, val)] = t.ap()
    nc.all_engine_barrier()

    # ---- I/O ----
    xT = nc.dram_tensor("xT", [HT, 128, S], F32R, kind="ExternalInput")
    wq = nc.dram_tensor("wq", [HT, 128, QKV_COLS], F32R, kind="ExternalInput")
    wp = nc.dram_tensor("wp", [HPC, 128, HID], F32R, kind="ExternalInput")
    cosq = nc.dram_tensor("cosq", [128, S], F32, kind="ExternalInput")
    sinq = nc.dram_tensor("sinq", [128, S], F32, kind="ExternalInput")
    cosk = nc.dram_tensor("cosk", [128, S], F32, kind="ExternalInput")
    sink = nc.dram_tensor("sink", [128, S], F32, kind="ExternalInput")
    ones_col = nc.dram_tensor("ones_col", [128, 1], F32R, kind="ExternalInput")
    ident = nc.dram_tensor("ident", [128, 128], F32R, kind="ExternalInput")
    out = nc.dram_tensor("out", [S, HID], F32, kind="ExternalOutput")

    with tile.TileContext(nc) as tc:
      for _rep in range(repeat):
        with tc.tile_pool(name=f"persist{_rep}", bufs=1) as pers, \
             tc.tile_pool(name=f"cols{_rep}", bufs=1) as cols, \
             tc.tile_pool(name=f"scr{_rep}", bufs=1, space="DRAM") as dscr:
            # DRAM row bounces (for partition-broadcast / row->col reshape)
            invr_scr = dscr.tile([1, S], F32, name="invr_scr")
            rrow_scr = dscr.tile([1, S], F32, name="rrow_scr")
            nq_scr = dscr.tile([HPC, S], F32, name="nq_scr")
            nk_scr = dscr.tile([1, S], F32, name="nk_scr")
            gate_scr = dscr.tile([HPC, S], F32, name="gate_scr")
            # persistent tiles
            t_ones = cols.tile([128, 1], F32R, name="ones")
            nc.sync.dma_start(t_ones[:, :], ones_col[:, :])
            t_id = cols.tile([128, 128], F32R, name="ident")
            nc.sync.dma_start(t_id[:, :], ident[:, :])

            q_t = [pers.tile([128, S], F32R, name=f"q{h}") for h in range(HPC)]
            k_t = pers.tile([128, S], F32R, name="kT")
            v_t = pers.tile([128, S], F32R, name="vT")
            vnat = pers.tile([128, S], F32R, name="vnat")
            gates = pers.tile([5, S], F32, name="gates")
            o_t = [pers.tile([128, S], F32R, name=f"o{h}") for h in range(HPC)]

            # ================= Phase 1: QKV matmul (+ x^2 accumulation) ========
            with tc.tile_pool(name=f"qkv_ps{_rep}", bufs=1, space="PSUM") as psA, \
                 tc.tile_pool(name=f"qkv_psB{_rep}", bufs=1, space="PSUM") as psB, \
                 tc.tile_pool(name=f"qkv_sb{_rep}", bufs=2) as sb2, \
                 tc.tile_pool(name=f"qkv_sb3{_rep}", bufs=3) as sb3:
                for ch in range(NCH):
                    c0 = ch * 512
                    pm = [psA.tile([128, 512], F32, name=f"mt{m}") for m in range(7)]
                    pg = psB.tile([5, 512], F32, name="small")
                    acc = sb2.tile([128, 512], F32, name="accsq")
                    accr = sb2.tile([128, 512], F32R, name="accr")
                    for ht in range(HT):
                        xt = sb2.tile([128, 512], F32R, name="xt")
                        nc.sync.dma_start(xt[:, :], xT[ht, :, c0:c0 + 512])
                        ws = sb2.tile([128, QKV_COLS], F32R, name="wslab")
                        nc.sync.dma_start(ws[:, :], wq[ht, :, :])
                        for m in range(7):
                            nc.tensor.matmul(pm[m][:, :], ws[:, m * 128:(m + 1) * 128],
                                             xt[:, :], start=(ht == 0), stop=(ht == HT - 1))
                        nc.tensor.matmul(pg[:, :], ws[:, 896:901], xt[:, :],
                                         start=(ht == 0), stop=(ht == HT - 1))
                        # x^2 accumulation (for pre-norm rms)
                        sq = sb3.tile([128, 512], F32, name="sq")
                        nc.scalar.activation(sq[:, :], xt[:, :].bitcast(F32), AF.Square)
                        if ht == 0:
                            nc.gpsimd.tensor_copy(acc[:, :], sq[:, :])
                        elif ht < HT - 1:
                            nc.gpsimd.tensor_add(acc[:, :], acc[:, :], sq[:, :])
                        else:
                            # last add on DVE, writes rounded f32r tile
                            nc.vector.tensor_add(accr[:, :], acc[:, :], sq[:, :])
                    # drain psums
                    for h in range(HPC):
                        nc.vector.tensor_copy(q_t[h][:, c0:c0 + 512], pm[h][:, :])
                    nc.vector.tensor_copy(k_t[:, c0:c0 + 512], pm[5][:, :])
                    nc.vector.tensor_copy(v_t[:, c0:c0 + 512], pm[6][:, :])
                    nc.vector.tensor_copy(gates[:, c0:c0 + 512], pg[:, :])
                    # r = 1/sqrt(mean_h x^2 + eps); bounce rows to DRAM
                    pr = psB.tile([1, 512], F32, name="small")
                    nc.tensor.matmul(pr[:, :], t_ones[:, :], accr[:, :],
                                     start=True, stop=True)
                    invr_row = sb2.tile([1, 512], F32, name="invr_row")
                    nc.scalar.activation(invr_row[:, :], pr[:, :], AF.Sqrt,
                                         bias=EPS, scale=1.0 / HID)
                    nc.sync.dma_start(invr_scr[0:1, c0:c0 + 512], invr_row[:, :])
                    r_row = sb2.tile([1, 512], F32, name="r_row")
                    nc.vector.reciprocal(r_row[:, :], invr_row[:, :])
                    nc.sync.dma_start(rrow_scr[0:1, c0:c0 + 512], r_row[:, :])

            # ================= Phase 2: prep (gates, v transpose, k/q norm+rope)
            with tc.tile_pool(name=f"at_sc{_rep}", bufs=3, space="PSUM") as pSC, \
                 tc.tile_pool(name=f"at_av{_rep}", bufs=2, space="PSUM") as pAV, \
                 tc.tile_pool(name=f"at_row{_rep}", bufs=2, space="PSUM") as pRow, \
                 tc.tile_pool(name=f"at_tr{_rep}", bufs=1, space="PSUM") as pTr, \
                 tc.tile_pool(name=f"at_sb2{_rep}", bufs=2) as asb2, \
                 tc.tile_pool(name=f"at_sb3{_rep}", bufs=3) as asb3:

                # gates: g = sigmoid(r * g_raw)
                for ch in range(NCH):
                    c0 = ch * 512
                    rb = asb2.tile([5, 512], F32, name="rhatb")
                    nc.sync.dma_start(rb[:, :],
                                      rrow_scr[0:1, c0:c0 + 512].to_broadcast((5, 512)))
                    nc.vector.tensor_mul(gates[:, c0:c0 + 512],
                                         gates[:, c0:c0 + 512], rb[:, :])
                nc.scalar.activation(gates[:, :], gates[:, :], AF.Sigmoid)
                nc.sync.dma_start(gate_scr[:, :], gates[:, :])

                # v transpose: vnat[:, kt*128:(kt+1)*128] = v block kt (natural [kpos, D])
                for kt in range(KT):
                    ptr = pTr.tile([128, 128], F32R, name="tr")
                    nc.tensor.transpose(ptr[:, :], v_t[:, kt * 128:(kt + 1) * 128],
                                        t_id[:, :])
                    nc.vector.tensor_copy(vnat[:, kt * 128:(kt + 1) * 128], ptr[:, :])

                # ---- k: norm sums, rope (no n_k application; folded into exp scale)
                for ch in range(NCH):
                    c0 = ch * 512
                    ksq = asb3.tile([128, 512], F32R, name="sqc")
                    nc.scalar.activation(ksq[:, :], k_t[:, c0:c0 + 512].bitcast(F32),
                                         AF.Square)
                    pn = pRow.tile([1, 512], F32, name="nrow")
                    nc.tensor.matmul(pn[:, :], t_ones[:, :], ksq[:, :],
                                     start=True, stop=True)
                    nk_row = asb2.tile([1, 512], F32, name="nkrow")
                    # 1/(n_k/sqrt(D)) = sqrt(sum + D*eps)
                    nc.scalar.activation(nk_row[:, :], pn[:, :], AF.Sqrt,
                                         bias=D * EPS, scale=1.0)
                    nc.sync.dma_start(nk_scr[0:1, c0:c0 + 512], nk_row[:, :])

                t_cos = pers.tile([128, S], F32, name="cosT")
                t_sin = pers.tile([128, S], F32, name="sinT")
                nc.sync.dma_start(t_cos[:, :], cosk[:, :])
                nc.sync.dma_start(t_sin[:, :], sink[:, :])
                for ch in range(NCH):
                    c0 = ch * 512
                    rot = asb2.tile([128, 512], F32, name="rot")
                    nc.sync.dma_start(rot[0:64, :],
                                      k_t[64:128, c0:c0 + 512].bitcast(F32))
                    nc.sync.dma_start(rot[64:128, :],
                                      k_t[0:64, c0:c0 + 512].bitcast(F32))
                    t1 = asb2.tile([128, 512], F32, name="t1")
                    nc.vector.tensor_mul(t1[:, :], k_t[:, c0:c0 + 512].bitcast(F32),
                                         t_cos[:, c0:c0 + 512])
                    nc.vector.tensor_mul(rot[:, :], rot[:, :], t_sin[:, c0:c0 + 512])
                    nc.vector.tensor_add(k_t[:, c0:c0 + 512], t1[:, :], rot[:, :])

                # ---- q heads: norm sums -> nq rows; rope; apply n_q
                for h in range(HPC):
                    for ch in range(NCH):
                        c0 = ch * 512
                        qsq = asb3.tile([128, 512], F32R, name="sqc")
                        nc.scalar.activation(qsq[:, :],
                                             q_t[h][:, c0:c0 + 512].bitcast(F32),
                                             AF.Square)
                        pn = pRow.tile([1, 512], F32, name="nrow")
                        nc.tensor.matmul(pn[:, :], t_ones[:, :], qsq[:, :],
                                         start=True, stop=True)
                        sm = asb2.tile([1, 512], F32, name="sqrtm")
                        nc.scalar.activation(sm[:, :], pn[:, :], AF.Sqrt,
                                             bias=EPS, scale=1.0 / D)
                        nq_row = asb2.tile([1, 512], F32, name="nqrow")
                        nc.vector.reciprocal(nq_row[:, :], sm[:, :])
                        nc.sync.dma_start(nq_scr[h:h + 1, c0:c0 + 512], nq_row[:, :])

                # rope tables for q (overwrites k tables; waits for k-rope reads)
                nc.sync.dma_start(t_cos[:, :], cosq[:, :])
                nc.sync.dma_start(t_sin[:, :], sinq[:, :])
                for h in range(HPC):
                    for ch in range(NCH):
                        c0 = ch * 512
                        rot = asb2.tile([128, 512], F32, name="rot")
                        nc.sync.dma_start(rot[0:64, :],
                                          q_t[h][64:128, c0:c0 + 512].bitcast(F32))
                        nc.sync.dma_start(rot[64:128, :],
                                          q_t[h][0:64, c0:c0 + 512].bitcast(F32))
                        t1 = asb2.tile([128, 512], F32, name="t1")
                        nc.vector.tensor_mul(t1[:, :],
                                             q_t[h][:, c0:c0 + 512].bitcast(F32),
                                             t_cos[:, c0:c0 + 512])
                        nc.vector.tensor_mul(rot[:, :], rot[:, :],
                                             t_sin[:, c0:c0 + 512])
                        nb = asb2.tile([128, 512], F32, name="nb")
                        nc.sync.dma_start(
                            nb[:, :],
                            nq_scr[h:h + 1, c0:c0 + 512].to_broadcast((128, 512)))
                        nc.vector.tensor_add(t1[:, :], t1[:, :], rot[:, :])
                        nc.vector.tensor_mul(q_t[h][:, c0:c0 + 512], t1[:, :], nb[:, :])

                # exp scale/bias columns: nk_col = 1/nk_row (col), lnr = -ln(invr)
                nkc_raw = cols.tile([128, KT], F32, name="nkc_raw")
                nc.sync.dma_start(nkc_raw[:, :],
                                  nk_scr[0, :].rearrange("(t p) -> p t", p=128))
                nk_col = cols.tile([128, KT], F32, name="nk_col")
                nc.vector.reciprocal(nk_col[:, :], nkc_raw[:, :])
                invr_col = cols.tile([128, KT], F32, name="invr_col")
                nc.sync.dma_start(invr_col[:, :],
                                  invr_scr[0, :].rearrange("(t p) -> p t", p=128))
                lnr_col = cols.tile([128, KT], F32, name="lnr_col")
                nc.scalar.activation(lnr_col[:, :], invr_col[:, :], AF.Ln)
                nc.vector.tensor_scalar_mul(lnr_col[:, :], lnr_col[:, :], -1.0)
                rinv_col = cols.tile([128, KT], F32R, name="rinv_col")
                nc.vector.tensor_copy(rinv_col[:, :], invr_col[:, :])

                # ================= Phase 3: attention ======================
                for h in range(HPC):
                    for ch in range(NCH):
                        c0 = ch * 512
                        po = pAV.tile([128, 512], F32, name="av")
                        psum_row = pRow.tile([1, 512], F32, name="nrow")
                        for kt in range(KT):
                            k0 = kt * 128
                            ps = pSC.tile([128, 512], F32, name="sc")
                            nc.tensor.matmul(ps[:, :], k_t[:, k0:k0 + 128],
                                             q_t[h][:, c0:c0 + 512],
                                             start=True, stop=True)
                            et = asb3.tile([128, 512], F32R, name="expt")
                            nc.scalar.activation(et[:, :], ps[:, :], AF.Exp,
                                                 bias=lnr_col[:, kt:kt + 1],
                                                 scale=nk_col[:, kt:kt + 1])
                            nc.tensor.matmul(po[:, :], vnat[:, k0:k0 + 128], et[:, :],
                                             start=(kt == 0), stop=(kt == KT - 1))
                            nc.tensor.matmul(psum_row[:, :], rinv_col[:, kt:kt + 1],
                                             et[:, :], start=(kt == 0),
                                             stop=(kt == KT - 1))
                        # scale = gate / sum ; outT = po * scale
                        rcp = asb2.tile([1, 512], F32, name="rcp")
                        nc.vector.reciprocal(rcp[:, :], psum_row[:, :])
                        grow = asb2.tile([1, 512], F32, name="grow")
                        nc.sync.dma_start(grow[:, :], gate_scr[h:h + 1, c0:c0 + 512])
                        nc.vector.tensor_mul(rcp[:, :], rcp[:, :], grow[:, :])
                        sb = asb2.tile([128, 512], F32, name="scaleb")
                        nc.gpsimd.partition_broadcast(sb[:, :], rcp[:, :])
                        nc.vector.tensor_mul(o_t[h][:, c0:c0 + 512], po[:, :],
                                             sb[:, :])

            # ================= Phase 4: projection =====================
            with tc.tile_pool(name=f"pj_ps{_rep}", bufs=4, space="PSUM") as pPJ, \
                 tc.tile_pool(name=f"pj_sb{_rep}", bufs=2) as pjs:
                NT = HID // 512  # 10
                for nt in range(NT):
                    n0 = nt * 512
                    wt = [pjs.tile([128, 512], F32R, name=f"wp{h}") for h in range(HPC)]
                    for h in range(HPC):
                        nc.sync.dma_start(wt[h][:, :], wp[h, :, n0:n0 + 512])
                    for st in range(ST):
                        s0 = st * 128
                        pp = pPJ.tile([128, 512], F32, name="pj")
                        for h in range(HPC):
                            nc.tensor.matmul(pp[:, :], o_t[h][:, s0:s0 + 128],
                                             wt[h][:, :], start=(h == 0),
                                             stop=(h == HPC - 1))
                        ob = pjs.tile([128, 512], F32, name="outsb")
                        nc.vector.tensor_copy(ob[:, :], pp[:, :])
                        nc.sync.dma_start(out[s0:s0 + 128, n0:n0 + 512], ob[:, :])
    nc.finalize()
    return nc


# ---------------- host-side prep & execution ----------------

_CACHE = {}


def _get_exec(repeat=1):
    if repeat in _CACHE:
        return _CACHE[repeat]

    import jax
    from concourse import bass2jax, mybir as mb
    from jax.experimental.shard_map import shard_map
    from jax.sharding import Mesh, PartitionSpec

    bass2jax.install_neuronx_cc_hook()
    nc = build_program(repeat)

    part_name = nc.partition_id_tensor.name if nc.partition_id_tensor else None
    in_names, out_names, out_avals = [], [], []
    for alloc in nc.m.functions[0].allocations:
        if not isinstance(alloc, mb.MemoryLocationSet):
            continue
        name = alloc.memorylocations[0].name
        if alloc.kind == "ExternalInput":
            if name != part_name:
                in_names.append(name)
        elif alloc.kind == "ExternalOutput":
            out_names.append(name)
            out_avals.append(jax.core.ShapedArray(tuple(alloc.tensor_shape),
                                                  mb.dt.np(alloc.dtype)))
    n_params = len(in_names)
    all_names = in_names + out_names
    if part_name is not None:
        all_names = all_names + [part_name]

    def _body(*args):
        operands = list(args)
        if part_name is not None:
            operands.append(bass2jax.partition_id_tensor())
        outs = bass2jax._bass_exec_p.bind(
            *operands,
            out_avals=tuple(out_avals),
            in_names=tuple(all_names),
            out_names=tuple(out_names),
            lowering_input_output_aliases=(),
            sim_require_finite=True,
            sim_require_nnan=True,
            nc=nc,
        )
        return tuple(outs)

    devices = jax.devices()[:NC]
    mesh = Mesh(np.asarray(devices), ("core",))
    spec = (PartitionSpec("core"),) * (n_params + len(out_names))
    fn = jax.jit(shard_map(_body, mesh=mesh, in_specs=spec,
                           out_specs=(PartitionSpec("core"),) * len(out_names),
                           check_rep=False), keep_unused=True)
    _CACHE[repeat] = dict(fn=fn, nc=nc, in_names=in_names, out_names=out_names,
                          out_avals=out_avals, mesh=mesh)
    return _CACHE[repeat]


def prep_inputs(x, rope_cos, rope_sin, w_pre_norm, w_qkv, w_q_norm, w_k_norm,
                w_proj):
    """Build the per-core input dict list (host-side sharding/layout only)."""
    x = np.asarray(x, np.float32)
    w_qkv = np.asarray(w_qkv, np.float32)
    w_proj = np.asarray(w_proj, np.float32)
    w_pre = np.asarray(w_pre_norm, np.float32)
    w_qn = np.asarray(w_q_norm, np.float32)
    w_kn = np.asarray(w_k_norm, np.float32)
    cos = np.asarray(rope_cos, np.float32)[0]   # [S, D]
    sin = np.asarray(rope_sin, np.float32)[0]

    xT = np.ascontiguousarray(x[0].T).reshape(HT, 128, S)

    cosT = np.ascontiguousarray(cos.T)          # [D, S]
    sinT = np.ascontiguousarray(sin.T)
    sign = np.where(np.arange(D) < D // 2, -1.0, 1.0).astype(np.float32)

    def rope_tables(w):
        w_swap = np.concatenate([w[D // 2:], w[:D // 2]])
        c = cosT * w[:, None]
        s = sinT * (sign * w_swap)[:, None]
        return np.ascontiguousarray(c), np.ascontiguousarray(s)

    cq, sq_ = rope_tables(w_qn)
    ck, sk = rope_tables(w_kn)

    wqkv_eff = w_pre[:, None] * w_qkv           # fold pre-norm weight (exact)
    q_dim, k_dim = NQ * D, NKV * D
    ones = np.ones((128, 1), np.float32)
    ident = np.eye(128, dtype=np.float32)

    in_maps = []
    for c in range(NC):
        wslice = np.concatenate([
            wqkv_eff[:, (HPC * c) * D:(HPC * c + HPC) * D],
            wqkv_eff[:, q_dim + c * D:q_dim + (c + 1) * D],
            wqkv_eff[:, q_dim + k_dim + c * D:q_dim + k_dim + (c + 1) * D],
            wqkv_eff[:, q_dim + 2 * k_dim + HPC * c:q_dim + 2 * k_dim + HPC * (c + 1)],
        ], axis=1)                               # [HID, 901]
        wslice = np.ascontiguousarray(wslice).reshape(HT, 128, QKV_COLS)
        wpslice = np.ascontiguousarray(
            w_proj[(HPC * c) * D:(HPC * c + HPC) * D, :]).reshape(HPC, 128, HID)
        in_maps.append({
            "xT": xT, "wq": wslice, "wp": wpslice,
            "cosq": cq, "sinq": sq_, "cosk": ck, "sink": sk,
            "ones_col": ones, "ident": ident,
        })
    return in_maps


def run_in_maps(in_maps):
    """Execute the SPMD program; returns list of per-core {out: [S, HID]}."""
    cache = _get_exec()
    fn, in_names, out_names, out_avals = (cache["fn"], cache["in_names"],
                                          cache["out_names"], cache["out_avals"])
    concat_in = [np.concatenate([m[nm] for m in in_maps], axis=0)
                 for nm in in_names]
    zeros = [np.zeros((NC * a.shape[0], *a.shape[1:]), a.dtype) for a in out_avals]
    outs = fn(*concat_in, *zeros)
    res = []
    for c in range(NC):
        d = {}
        for i, nm in enumerate(out_names):
            shp = out_avals[i].shape
            d[nm] = np.asarray(outs[i]).reshape(NC, *shp)[c]
        res.append(d)
    return res


def kernel(**inputs):
    in_maps = prep_inputs(**inputs)
    res = run_in_maps(in_maps)
    total = res[0]["out"].astype(np.float32)
    for c in range(1, NC):
        total = total + res[c]["out"]
    return total.reshape(1, S, HID)
